# revision 1
# baseline (speedup 1.0000x reference)
"""Trainium2 Bass kernel for nn_CRF_79551384256937 (CRF negative-log-likelihood loss).

Strategy (data-parallel over batch, 16 sequences per core, 8 cores):
  Forward partition function as a *multiplicative* scan in [tag, batch] layout:
      P_{t+1} = (expM^T @ P_t) * exp(u_t - c*),   expM[k, j] = exp(trans[j, k])
  with c* = log(254) + 0.5 a constant stabilizer (keeps P bounded in fp32/bf16,
  no renormalization needed).  Per step: 4 [128,128]x[128,16] bf16 matmuls
  (PSUM f32 accumulate) + DVE multiply.  r_raw[t] = exp(trans[end,:]) . P_{t+1}
  accumulated into PSUM columns (32 steps per bank), logged in bulk at the end;
  fwd[b] = log(r_raw[len_b - 1, b]) + len_b * c*.
  Gold score: emissions via host-built one-hot mask O (elementwise mul + reduce
  of the same transposed-u tiles), transitions via host-built pair-count
  histogram CNT contracted with trans on the tensor engine.
All tag/length-derived index structures (one-hots, counts, masks) are prepared
on host; every floating-point reduction over model data runs on device.
"""
import os
import numpy as np
import ml_dtypes
from contextlib import ExitStack

import concourse.bass as bass
import concourse.bacc as bacc
import concourse.tile as tile
from concourse import mybir
from concourse.bass import MemorySpace
from concourse.bass_utils import run_bass_kernel_spmd

BF = ml_dtypes.bfloat16
F32 = np.float32

N_CORES = 8
B, T, NT = 128, 1024, 254
N = NT + 2            # 256 tags incl <GO>/<EOS>
BL = B // N_CORES     # 16 sequences per core
TC = 128              # time steps per chunk
NCH = T // TC         # 8 chunks
NEG = -10000.0
CSTAR = float(np.log(254.0) + 0.5)
GRP = 32              # r-row steps per PSUM bank
NGRP = T // GRP       # 32 groups

_compiled = {}


def _build_nc():
    nc = bacc.Bacc("TRN2", target_bir_lowering=False, debug=False,
                   num_devices=N_CORES)
    dt = mybir.dt
    # ---- DRAM I/O (per-core shapes) ----
    u_pad = nc.dram_tensor("u_pad", [T * BL, N], dt.bfloat16, kind="ExternalInput").ap()
    O_in = nc.dram_tensor("onehot", [N, T * BL], dt.bfloat16, kind="ExternalInput").ap()
    cnt_in = nc.dram_tensor("cnt", [128, 512 * BL], dt.float32, kind="ExternalInput").ap()
    transT_in = nc.dram_tensor("transT", [N, N], dt.float32, kind="ExternalInput").ap()
    tg_in = nc.dram_tensor("trans_gold", [128, 512 * BL], dt.float32, kind="ExternalInput").ap()
    p0_in = nc.dram_tensor("p0", [N, BL], dt.bfloat16, kind="ExternalInput").ap()
    msel_in = nc.dram_tensor("msel", [NGRP, GRP * BL], dt.float32, kind="ExternalInput").ap()
    lenc_in = nc.dram_tensor("lenc", [1, BL], dt.float32, kind="ExternalInput").ap()
    ones_in = nc.dram_tensor("ones", [128, 128], dt.float32, kind="ExternalInput").ap()
    out_d = nc.dram_tensor("out", [1, BL], dt.float32, kind="ExternalOutput").ap()

    with tile.TileContext(nc) as tc:
        with ExitStack() as ctx:
            singles = ctx.enter_context(tc.tile_pool(name="singles", bufs=1))
            chunks = ctx.enter_context(tc.tile_pool(name="chunks", bufs=2))
            ppool = ctx.enter_context(tc.tile_pool(name="ppool", bufs=3))
            spsum = ctx.enter_context(
                tc.tile_pool(name="spsum", bufs=4, space=MemorySpace.PSUM))
            gpsum = ctx.enter_context(
                tc.tile_pool(name="gpsum", bufs=1, space=MemorySpace.PSUM))

            # ---- constants / singles ----
            tT = [singles.tile([128, N], dt.float32, name=f"tT{h}") for h in (0, 1)]
            expM = [singles.tile([128, N], dt.bfloat16, name=f"expM{h}") for h in (0, 1)]
            for h in (0, 1):
                nc.sync.dma_start(out=tT[h], in_=transT_in[128 * h:128 * (h + 1), :])
                nc.scalar.activation(out=expM[h], in_=tT[h],
                                     func=mybir.ActivationFunctionType.Exp)
            Pinit = singles.tile([128, 2 * BL], dt.bfloat16)
            for h in (0, 1):
                nc.sync.dma_start(out=Pinit[:, BL * h:BL * (h + 1)],
                                  in_=p0_in[128 * h:128 * (h + 1), :])
            cnt_sb = singles.tile([128, 512 * BL], dt.float32)
            nc.sync.dma_start(out=cnt_sb, in_=cnt_in)
            tg_sb = singles.tile([128, 512 * BL], dt.float32)
            nc.sync.dma_start(out=tg_sb, in_=tg_in)
            ones_sb = singles.tile([128, 128], dt.float32)
            nc.sync.dma_start(out=ones_sb, in_=ones_in)
            msel_sb = singles.tile([NGRP, GRP * BL], dt.float32)
            nc.sync.dma_start(out=msel_sb, in_=msel_in)
            lenc_sb = singles.tile([1, BL], dt.float32)
            nc.sync.dma_start(out=lenc_sb, in_=lenc_in)
            gacc = singles.tile([128, BL], dt.float32)
            cbias = singles.tile([128, 1], dt.float32)
            nc.vector.memset(cbias, -CSTAR)
            nc.vector.memset(gacc, 0.0)
            rbuf = singles.tile([NGRP, GRP * BL], dt.float32)


            # ---- the scan ----
            # r_raw[tau] = eEnd . P_{tau+1} = row 255 of S_{tau+1} (j=255 is a
            # dead pad row: its P is always zeroed by eU), extracted with an
            # ACT copy from S PSUM partition 127 of the g=1 half.
            Pprev = None  # set to [PinitA, PinitB] below
            stg = None

            def extract_r(S, tau):
                nonlocal stg
                g, sl = tau // GRP, tau % GRP
                if sl == 0:
                    stg = ppool.tile([32, GRP * BL], dt.float32, tag="rstg")
                nc.scalar.copy(out=stg[:, BL * sl:BL * (sl + 1)],
                               in_=S[96:128, BL:2 * BL])
                if sl == GRP - 1:
                    nc.sync.dma_start(out=rbuf[g:g + 1, :], in_=stg[31:32, :])

            def chunk_loads(ch):
                uT = chunks.tile([128, 2 * TC * BL], dt.bfloat16, tag="uT",
                                 name=f"uT{ch}")
                for h in (0, 1):
                    nc.sync.dma_start_transpose(
                        out=uT[:, TC * BL * h:TC * BL * (h + 1)],
                        in_=u_pad[ch * TC * BL:(ch + 1) * TC * BL,
                                  128 * h:128 * (h + 1)])
                eU = chunks.tile([128, 2 * TC * BL], dt.bfloat16, tag="eU",
                                 name=f"eU{ch}")
                nc.scalar.activation(
                    out=eU[:, :].rearrange("p (s h b) -> p h s b", h=2, b=BL),
                    in_=uT[:, :].rearrange("p (h s b) -> p h s b", h=2, b=BL),
                    func=mybir.ActivationFunctionType.Exp,
                    bias=cbias[:, :])
                Ot = chunks.tile([128, 2 * TC * BL], dt.bfloat16, tag="Ot",
                                 name=f"Ot{ch}")
                for h in (0, 1):
                    nc.sync.dma_start(
                        out=Ot[:, TC * BL * h:TC * BL * (h + 1)],
                        in_=O_in[128 * h:128 * (h + 1),
                                 ch * TC * BL:(ch + 1) * TC * BL])
                gp = chunks.tile([128, 2 * TC * BL], dt.bfloat16, tag="gp",
                                 name=f"gp{ch}")
                for q in range(4):
                    sl = slice(1024 * q, 1024 * (q + 1))
                    nc.gpsimd.tensor_mul(gp[:, sl], Ot[:, sl], uT[:, sl])
                return eU, gp

            def gold_piece(gp, piece):
                src = gp[:, 256 * piece:256 * (piece + 1)].rearrange(
                    "p (s b) -> p b s", b=BL)
                rtmp = ppool.tile([128, BL], dt.float32, tag="rtmp")
                nc.vector.tensor_reduce(rtmp, src, axis=mybir.AxisListType.X,
                                        op=mybir.AluOpType.add)
                nc.vector.tensor_add(gacc, gacc, rtmp)

            Pprev = Pinit
            loads = {0: chunk_loads(0)}
            for ch in range(NCH):
                eU, gp = loads.pop(ch)
                for s in range(TC):
                    t = ch * TC + s
                    S = spsum.tile([128, 2 * BL], dt.float32, tag="S")
                    for g in (0, 1):
                        for h in (0, 1):
                            nc.tensor.matmul(
                                S[:, BL * g:BL * (g + 1)],
                                expM[h][:, 128 * g:128 * (g + 1)],
                                Pprev[:, BL * h:BL * (h + 1)],
                                start=(h == 0), stop=(h == 1))
                    Pn = ppool.tile([128, 2 * BL], dt.bfloat16, tag="P")
                    nc.vector.tensor_mul(
                        Pn, S, eU[:, 2 * BL * s:2 * BL * (s + 1)])
                    if t > 0:
                        extract_r(S, t - 1)
                    if s == 8 and ch + 1 < NCH:
                        loads[ch + 1] = chunk_loads(ch + 1)
                    if s % 8 == 5 and s // 8 < 16:
                        gold_piece(gp, s // 8)
                    Pprev = Pn
            # tail: S_{1024} g=1 half only, to extract r_raw[1023]
            Sx = spsum.tile([128, 2 * BL], dt.float32, tag="S")
            for h in (0, 1):
                nc.tensor.matmul(Sx[:, BL:2 * BL],
                                 expM[h][:, 128:256],
                                 Pprev[:, BL * h:BL * (h + 1)],
                                 start=(h == 0), stop=(h == 1))
            extract_r(Sx, T - 1)

            # ---- gold transition score (after scan; overlaps the tail) ----
            gtp = singles.tile([128, 512 * BL], dt.float32)
            for q in range(4):
                sl = slice(2048 * q, 2048 * (q + 1))
                nc.gpsimd.tensor_mul(gtp[:, sl], cnt_sb[:, sl], tg_sb[:, sl])
            for piece in range(8):
                src = gtp[:, 1024 * piece:1024 * (piece + 1)].rearrange(
                    "p (c b) -> p b c", b=BL)
                rtmp = ppool.tile([128, BL], dt.float32, tag="rtmp", name="rtg")
                nc.vector.tensor_reduce(rtmp, src, axis=mybir.AxisListType.X,
                                        op=mybir.AluOpType.add)
                nc.vector.tensor_add(gacc, gacc, rtmp)

            # ---- final assembly ----
            rlog = singles.tile([NGRP, GRP * BL], dt.float32)
            nc.scalar.activation(out=rlog, in_=rbuf,
                                 func=mybir.ActivationFunctionType.Ln)
            rm = singles.tile([NGRP, GRP * BL], dt.float32)
            nc.vector.tensor_mul(rm, rlog, msel_sb)
            rsum = singles.tile([NGRP, BL], dt.float32)
            nc.vector.tensor_reduce(
                rsum, rm.rearrange("p (s b) -> p b s", b=BL),
                axis=mybir.AxisListType.X, op=mybir.AluOpType.add)
            rsel_ps = gpsum.tile([128, BL], dt.float32, tag="rsel")
            nc.tensor.matmul(rsel_ps, ones_sb[0:NGRP, :], rsum, start=True, stop=True)
            ge_ps = gpsum.tile([128, BL], dt.float32, tag="ge")
            nc.tensor.matmul(ge_ps, ones_sb, gacc, start=True, stop=True)

            x1 = singles.tile([1, BL], dt.float32, tag="x1")
            nc.vector.tensor_add(x1, rsel_ps[0:1, :], lenc_sb)
            x3 = singles.tile([1, BL], dt.float32, tag="x3")
            nc.vector.tensor_sub(x3, x1, ge_ps[0:1, :])
            nc.sync.dma_start(out=out_d, in_=x3)

    nc.compile()
    return nc


def _host_prep(unary, tags, lengths, transitions):
    """Build the 8 per-core input maps (index prep + layout only)."""
    unary = np.asarray(unary, dtype=F32)
    tags = np.asarray(tags).astype(np.int64)
    lengths = np.asarray(lengths).astype(np.int64)
    trans = np.asarray(transitions, dtype=F32)

    transT = np.ascontiguousarray(trans.T)
    trans_flat = trans.reshape(-1)
    trans_gold = np.ascontiguousarray(
        np.repeat(trans_flat.reshape(512, 128).T, BL, axis=1))
    ones = np.ones((128, 128), dtype=F32)

    in_maps = []
    for c in range(N_CORES):
        sl = slice(c * BL, (c + 1) * BL)
        u = unary[sl]          # [16, 1024, 254]
        tg = tags[sl]          # [16, 1024]
        ln = lengths[sl]       # [16]

        u_pad = np.full((T, BL, N), NEG, dtype=BF)
        u_pad[:, :, :NT] = np.transpose(u, (1, 0, 2)).astype(BF)

        tmask = np.arange(T)[None, :] < ln[:, None]
        tg_m = np.where(tmask, tg, 300)
        O = (np.arange(N)[:, None, None] == tg_m.T[None, :, :]).astype(BF)

        cnt = np.zeros((N * N, BL), dtype=F32)
        prev = np.concatenate([np.full((BL, 1), NT, dtype=np.int64),
                               tg[:, :-1]], axis=1)
        flat = (tg * N + prev)  # [16, 1024]
        for b in range(BL):
            np.add.at(cnt[:, b], flat[b, :ln[b]], 1.0)
            last = tg[b, ln[b] - 1]
            cnt[(NT + 1) * N + last, b] += 1.0
        cnt_dev = np.ascontiguousarray(
            cnt.reshape(512, 128, BL).transpose(1, 0, 2).reshape(128, 512 * BL))

        p0 = np.zeros((N, BL), dtype=BF)
        p0[NT, :] = 1.0

        msel = np.zeros((NGRP, GRP * BL), dtype=F32)
        for b in range(BL):
            tsel = int(ln[b]) - 1
            msel[tsel // GRP, (tsel % GRP) * BL + b] = 1.0

        lenc = (ln.astype(F32) * CSTAR).reshape(1, BL)

        in_maps.append({
            "u_pad": np.ascontiguousarray(u_pad.reshape(T * BL, N)),
            "onehot": np.ascontiguousarray(O.reshape(N, T * BL)),
            "cnt": cnt_dev,
            "transT": transT,
            "trans_gold": trans_gold,
            "p0": p0,
            "msel": msel,
            "lenc": lenc,
            "ones": ones,
        })
    return in_maps


def kernel(unary, tags, lengths, transitions):
    if "nc" not in _compiled:
        _compiled["nc"] = _build_nc()
    nc = _compiled["nc"]
    in_maps = _host_prep(unary, tags, lengths, transitions)
    import os
    trace = bool(os.environ.get("CRF_TRACE"))
    res = run_bass_kernel_spmd(nc, in_maps, core_ids=list(range(N_CORES)),
                               trace=trace)
    if trace:
        _compiled["last_result"] = res
    out = np.concatenate([res.results[c]["out"].reshape(BL) for c in range(N_CORES)])
    return out.astype(F32)



# revision 5
# speedup vs baseline: 1.1116x; 1.1116x over previous
"""Trainium2 Bass kernel for nn_CRF_79551384256937 (CRF negative-log-likelihood loss).

Strategy (data-parallel over batch, 16 sequences per core, 8 cores):
  Forward partition function as a *multiplicative* scan in [tag, batch] layout:
      P_{t+1} = (expM^T @ P_t) * exp(u_t - c*),   expM[k, j] = exp(trans[j, k])
  with c* = log(254) + 0.5 a constant stabilizer (keeps P bounded in fp32/bf16,
  no renormalization needed).  Per step: 4 [128,128]x[128,16] bf16 matmuls
  (PSUM f32 accumulate) + one DVE multiply.  The critical path per step is
  MM-drain -> sem -> DVE -> sem -> next MMs; everything else is kept off it:

  * r extraction rides in the dead EOS lane: the expM column for k=EOS is
    zeroed (host-side) and eU[EOS] is forced to 1, so Pn[EOS] = S[EOS] =
    the forward-score numerator.  Pn tiles live in a 64-slot SBUF ring and
    a DMA on the (idle) sync queue harvests partition 127 every 32 steps.
    No per-step ACT copy, no extra semaphores on the matmuls.
  Gold score: emissions via host-built one-hot mask O (elementwise mul on
  GpSimd + DVE reduces interleaved into scan slack), transitions via
  host-built pair-count histogram CNT contracted with trans likewise.
All tag/length-derived index structures (one-hots, counts, masks) are prepared
on host; every floating-point reduction over model data runs on device.
"""
import os
import numpy as np
import ml_dtypes
from contextlib import ExitStack

import concourse.bass as bass
import concourse.bacc as bacc
import concourse.tile as tile
from concourse import mybir
from concourse.bass import MemorySpace
from concourse.bass_utils import run_bass_kernel_spmd

BF = ml_dtypes.bfloat16
F32 = np.float32

N_CORES = 8
B, T, NT = 128, 1024, 254
N = NT + 2            # 256 tags incl <GO>/<EOS>
BL = B // N_CORES     # 16 sequences per core
TC = 128              # time steps per chunk
NCH = T // TC         # 8 chunks
NEG = -10000.0
CSTAR = float(np.log(254.0) + 0.5)
GRP = 32              # r harvest window
NGRP = T // GRP       # 32 windows
NSLOT = 64            # Pn ring slots

_compiled = {}


def _build_nc():
    nc = bacc.Bacc("TRN2", target_bir_lowering=False, debug=False,
                   num_devices=N_CORES)
    dt = mybir.dt
    # ---- DRAM I/O (per-core shapes) ----
    u_pad = nc.dram_tensor("u_pad", [T * BL, N], dt.bfloat16, kind="ExternalInput").ap()
    O_in = nc.dram_tensor("onehot", [N, T * BL], dt.bfloat16, kind="ExternalInput").ap()
    cnt_in = nc.dram_tensor("cnt", [128, 512 * BL], dt.float32, kind="ExternalInput").ap()
    transT_in = nc.dram_tensor("transT", [N, N], dt.float32, kind="ExternalInput").ap()
    tg_in = nc.dram_tensor("trans_gold", [128, 512 * BL], dt.float32, kind="ExternalInput").ap()
    p0_in = nc.dram_tensor("p0", [N, BL], dt.bfloat16, kind="ExternalInput").ap()
    msel_in = nc.dram_tensor("msel", [NGRP, GRP * BL], dt.float32, kind="ExternalInput").ap()
    lenc_in = nc.dram_tensor("lenc", [1, BL], dt.float32, kind="ExternalInput").ap()
    ones_in = nc.dram_tensor("ones", [128, 128], dt.float32, kind="ExternalInput").ap()
    out_d = nc.dram_tensor("out", [1, BL], dt.float32, kind="ExternalOutput").ap()

    with tile.TileContext(nc) as tc:
        with ExitStack() as ctx:
            singles = ctx.enter_context(tc.tile_pool(name="singles", bufs=1))
            chunks = ctx.enter_context(tc.tile_pool(name="chunks", bufs=2))
            work = ctx.enter_context(tc.tile_pool(name="work", bufs=2))
            spsum = ctx.enter_context(
                tc.tile_pool(name="spsum", bufs=2, space=MemorySpace.PSUM))
            gpsum = ctx.enter_context(
                tc.tile_pool(name="gpsum", bufs=1, space=MemorySpace.PSUM))

            # ---- constants / singles ----
            tT = [singles.tile([128, N], dt.float32, name=f"tT{h}") for h in (0, 1)]
            expM = [singles.tile([128, N], dt.bfloat16, name=f"expM{h}") for h in (0, 1)]
            for h in (0, 1):
                nc.sync.dma_start(out=tT[h], in_=transT_in[128 * h:128 * (h + 1), :])
                nc.scalar.activation(out=expM[h], in_=tT[h],
                                     func=mybir.ActivationFunctionType.Exp)
            # Pn ring: slot s = [:, 32*s : 32*s+32]; cols (g-half, b) per slot.
            PnR = singles.tile([128, NSLOT * 2 * BL], dt.bfloat16)
            # P_0 (GO one-hot) -> slot 62 (read-slot of round 0 = (0-2) % 64).
            for h in (0, 1):
                nc.sync.dma_start(out=PnR[:, 62 * 32 + BL * h:62 * 32 + BL * (h + 1)],
                                  in_=p0_in[128 * h:128 * (h + 1), :])
            cnt_sb = singles.tile([128, 512 * BL], dt.float32)
            nc.sync.dma_start(out=cnt_sb, in_=cnt_in)
            tg_sb = singles.tile([128, 512 * BL], dt.float32)
            nc.sync.dma_start(out=tg_sb, in_=tg_in)
            ones_sb = singles.tile([128, 128], dt.float32)
            nc.sync.dma_start(out=ones_sb, in_=ones_in)
            msel_sb = singles.tile([NGRP, GRP * BL], dt.float32)
            nc.sync.dma_start(out=msel_sb, in_=msel_in)
            lenc_sb = singles.tile([1, BL], dt.float32)
            nc.sync.dma_start(out=lenc_sb, in_=lenc_in)
            gacc = singles.tile([128, BL], dt.float32)
            cbias = singles.tile([128, 1], dt.float32)
            nc.vector.memset(cbias, -CSTAR)
            nc.vector.memset(gacc, 0.0)
            rbuf = singles.tile([NGRP, GRP * BL], dt.bfloat16)
            gtp = singles.tile([128, 512 * BL], dt.float32)

            def chunk_loads(ch):
                uT = chunks.tile([128, 2 * TC * BL], dt.bfloat16, tag="uT",
                                 name=f"uT{ch}")
                for h in (0, 1):
                    nc.sync.dma_start_transpose(
                        out=uT[:, TC * BL * h:TC * BL * (h + 1)],
                        in_=u_pad[ch * TC * BL:(ch + 1) * TC * BL,
                                  128 * h:128 * (h + 1)])
                eU = chunks.tile([128, 2 * TC * BL], dt.bfloat16, tag="eU",
                                 name=f"eU{ch}")
                nc.scalar.activation(
                    out=eU[:, :].rearrange("p (s h b) -> p h s b", h=2, b=BL),
                    in_=uT[:, :].rearrange("p (h s b) -> p h s b", h=2, b=BL),
                    func=mybir.ActivationFunctionType.Exp,
                    bias=cbias[:, :])
                Ot = chunks.tile([128, 2 * TC * BL], dt.bfloat16, tag="Ot",
                                 name=f"Ot{ch}")
                for h in (0, 1):
                    nc.sync.dma_start(
                        out=Ot[:, TC * BL * h:TC * BL * (h + 1)],
                        in_=O_in[128 * h:128 * (h + 1),
                                 ch * TC * BL:(ch + 1) * TC * BL])
                gp = chunks.tile([128, 2 * TC * BL], dt.bfloat16, tag="gp",
                                 name=f"gp{ch}")
                for q in range(4):
                    sl = slice(1024 * q, 1024 * (q + 1))
                    nc.gpsimd.tensor_mul(gp[:, sl], Ot[:, sl], uT[:, sl])
                return eU, gp

            # gold-emission partial: sum over the s-range of one chunk piece
            def gold_piece(gp, piece):
                src = gp[:, 256 * piece:256 * (piece + 1)].rearrange(
                    "p (s b) -> p b s", b=BL)
                rtmp = work.tile([128, BL], dt.float32, tag="rtmp")
                nc.vector.tensor_reduce(rtmp, src, axis=mybir.AxisListType.X,
                                        op=mybir.AluOpType.add)
                nc.vector.tensor_add(gacc, gacc, rtmp)

            # gold-transition partials, chopped into 16 pieces to hide in
            # the scan's Vector slack
            def gold_trans_mul(q):
                sl = slice(2048 * q, 2048 * (q + 1))
                nc.gpsimd.tensor_mul(gtp[:, sl], cnt_sb[:, sl], tg_sb[:, sl])

            def gold_trans_reduce(piece):
                src = gtp[:, 512 * piece:512 * (piece + 1)].rearrange(
                    "p (c b) -> p b c", b=BL)
                rtmp = work.tile([128, BL], dt.float32, tag="rtmp", name="rtg")
                nc.vector.tensor_reduce(rtmp, src, axis=mybir.AxisListType.X,
                                        op=mybir.AluOpType.add)
                nc.vector.tensor_add(gacc, gacc, rtmp)

            # ---- the scan ----
            def slot_cols(s, h=None):
                if h is None:
                    return slice(32 * s, 32 * s + 32)
                return slice(32 * s + BL * h, 32 * s + BL * (h + 1))

            loads = {0: chunk_loads(0)}
            for t in range(T):
                ch, s = t // TC, t % TC
                if s == 0:
                    eU, gp = loads.pop(ch)
                rd, wr = (t - 2) % NSLOT, (t - 1) % NSLOT
                S = spsum.tile([128, 2 * BL], dt.float32, tag="S")
                for g in (0, 1):
                    for h in (0, 1):
                        nc.tensor.matmul(
                            S[:, BL * g:BL * (g + 1)],
                            expM[h][:, 128 * g:128 * (g + 1)],
                            PnR[:, slot_cols(rd, h)],
                            start=(h == 0), stop=(h == 1))
                nc.vector.tensor_mul(
                    PnR[:, slot_cols(wr)], S,
                    eU[:, 2 * BL * s:2 * BL * (s + 1)])
                if s == 8 and ch + 1 < NCH:
                    loads[ch + 1] = chunk_loads(ch + 1)
                if s % 8 == 5 and s // 8 < 16:
                    gold_piece(gp, s // 8)
                if ch == 2 and s % 32 == 21:
                    gold_trans_mul(s // 32)
                if ch >= 4 and s % 16 == 13 and (ch - 4) * 8 + s // 16 < 16:
                    gold_trans_reduce((ch - 4) * 8 + s // 16)
                # harvest window w: rounds 32w+1..32w+32 wrote slots
                # 32w..32w+31 (mod 64); issue after round 32w+32's DVE.
                if t % GRP == 0 and t > 0:
                    w = t // GRP - 1
                    base = (GRP * w) % NSLOT
                    nc.sync.dma_start(
                        out=rbuf[w:w + 1, :],
                        in_=PnR[127:128, :]
                        .rearrange("p (s x) -> p s x", x=32)
                        [:, base:base + GRP, BL:2 * BL])

            # tail round 1024: S^(1024) g=1 half only -> slot 63 cols 16:32
            Sx = spsum.tile([128, 2 * BL], dt.float32, tag="S")
            for h in (0, 1):
                nc.tensor.matmul(Sx[:, BL:2 * BL],
                                 expM[h][:, 128:256],
                                 PnR[:, slot_cols((T - 2) % NSLOT, h)],
                                 start=(h == 0), stop=(h == 1))
            nc.vector.tensor_copy(out=PnR[:, slot_cols(63, 1)],
                                  in_=Sx[:, BL:2 * BL])
            w = NGRP - 1
            base = (GRP * w) % NSLOT
            nc.sync.dma_start(
                out=rbuf[w:w + 1, :],
                in_=PnR[127:128, :]
                .rearrange("p (s x) -> p s x", x=32)
                [:, base:base + GRP, BL:2 * BL])

            # ---- final assembly ----
            rlog = singles.tile([NGRP, GRP * BL], dt.float32)
            nc.scalar.activation(out=rlog, in_=rbuf,
                                 func=mybir.ActivationFunctionType.Ln)
            rm = singles.tile([NGRP, GRP * BL], dt.float32)
            nc.vector.tensor_mul(rm, rlog, msel_sb)
            rsum = singles.tile([NGRP, BL], dt.float32)
            nc.vector.tensor_reduce(
                rsum, rm.rearrange("p (s b) -> p b s", b=BL),
                axis=mybir.AxisListType.X, op=mybir.AluOpType.add)
            rsel_ps = gpsum.tile([128, BL], dt.float32, tag="rsel")
            nc.tensor.matmul(rsel_ps, ones_sb[0:NGRP, :], rsum, start=True, stop=True)
            ge_ps = gpsum.tile([128, BL], dt.float32, tag="ge")
            nc.tensor.matmul(ge_ps, ones_sb, gacc, start=True, stop=True)

            x1 = singles.tile([1, BL], dt.float32, tag="x1")
            nc.vector.tensor_add(x1, rsel_ps[0:1, :], lenc_sb)
            x3 = singles.tile([1, BL], dt.float32, tag="x3")
            nc.vector.tensor_sub(x3, x1, ge_ps[0:1, :])
            nc.sync.dma_start(out=out_d, in_=x3)

    nc.compile()
    return nc


def _host_prep(unary, tags, lengths, transitions):
    """Build the 8 per-core input maps (index prep + layout only)."""
    unary = np.asarray(unary, dtype=F32)
    tags = np.asarray(tags).astype(np.int64)
    lengths = np.asarray(lengths).astype(np.int64)
    trans = np.asarray(transitions, dtype=F32)

    transT = np.ascontiguousarray(trans.T)
    # zero the k=EOS column of M' (exp -> 0) so the EOS lane of Pn is a free
    # accumulator carrying S[EOS] (the fwd-score numerator)
    transT[NT + 1, :] = NEG
    trans_flat = trans.reshape(-1)
    trans_gold = np.ascontiguousarray(
        np.repeat(trans_flat.reshape(512, 128).T, BL, axis=1))
    ones = np.ones((128, 128), dtype=F32)

    in_maps = []
    for c in range(N_CORES):
        sl = slice(c * BL, (c + 1) * BL)
        u = unary[sl]          # [16, 1024, 254]
        tg = tags[sl]          # [16, 1024]
        ln = lengths[sl]       # [16]

        u_pad = np.full((T, BL, N), NEG, dtype=BF)
        u_pad[:, :, :NT] = np.transpose(u, (1, 0, 2)).astype(BF)
        u_pad[:, :, NT + 1] = CSTAR   # eU[EOS] = exp(c*-c*) = 1

        tmask = np.arange(T)[None, :] < ln[:, None]
        tg_m = np.where(tmask, tg, 300)
        O = (np.arange(N)[:, None, None] == tg_m.T[None, :, :]).astype(BF)

        cnt = np.zeros((N * N, BL), dtype=F32)
        prev = np.concatenate([np.full((BL, 1), NT, dtype=np.int64),
                               tg[:, :-1]], axis=1)
        flat = (tg * N + prev)  # [16, 1024]
        for b in range(BL):
            np.add.at(cnt[:, b], flat[b, :ln[b]], 1.0)
            last = tg[b, ln[b] - 1]
            cnt[(NT + 1) * N + last, b] += 1.0
        cnt_dev = np.ascontiguousarray(
            cnt.reshape(512, 128, BL).transpose(1, 0, 2).reshape(128, 512 * BL))

        p0 = np.zeros((N, BL), dtype=BF)
        p0[NT, :] = 1.0

        msel = np.zeros((NGRP, GRP * BL), dtype=F32)
        for b in range(BL):
            tsel = int(ln[b]) - 1
            msel[tsel // GRP, (tsel % GRP) * BL + b] = 1.0

        lenc = (ln.astype(F32) * CSTAR).reshape(1, BL)

        in_maps.append({
            "u_pad": np.ascontiguousarray(u_pad.reshape(T * BL, N)),
            "onehot": np.ascontiguousarray(O.reshape(N, T * BL)),
            "cnt": cnt_dev,
            "transT": transT,
            "trans_gold": trans_gold,
            "p0": p0,
            "msel": msel,
            "lenc": lenc,
            "ones": ones,
        })
    return in_maps


def kernel(unary, tags, lengths, transitions):
    if "nc" not in _compiled:
        _compiled["nc"] = _build_nc()
    nc = _compiled["nc"]
    in_maps = _host_prep(unary, tags, lengths, transitions)
    trace = bool(os.environ.get("CRF_TRACE"))
    res = run_bass_kernel_spmd(nc, in_maps, core_ids=list(range(N_CORES)),
                               trace=trace)
    if trace:
        _compiled["last_result"] = res
    out = np.concatenate([res.results[c]["out"].reshape(BL) for c in range(N_CORES)])
    return out.astype(F32)
